# revision 3
# baseline (speedup 1.0000x reference)
"""Single-head attention (B=8, S=4096, D=1024, DK=DV=128) on 8 TRN2 NeuronCores.

Sharding: data-parallel over batch — one batch element per core, the three
Linear weights replicated. No collectives.

Per-core pipeline (bf16 TensorE compute, f32 PSUM accumulation):
  1. q/k/v inputs cast-loaded f32->bf16 (SWDGE), transposed on-chip with the
     DMA xbar (d onto partitions) in [128, 1024] -> [128, 8, 128] calls.
  2. Projections: Q^T/K^T = Wq/Wk.T @ x^T (bias via per-partition DVE add),
     V in natural [s, dv] layout with a ones-column appended.
  3. Attention in transposed-score layout: S^T[k, q] = K^T_tile.T @ Q^T,
     exp on ScalarE (scores are bounded |s|<~2.5, no max-subtraction needed),
     P^T @ [V | 1] accumulated in PSUM — the ones-column yields the softmax
     denominator in the same matmul. Normalize + bias on VectorE
     (out = attn@V + bv exactly, since rows of attn sum to 1).
"""

import math

import numpy as np

B, S, D, DK, DV = 8, 4096, 1024, 128, 128
P = 128
SB = 512  # q/s block size
CD = D // P  # 8 d-chunks
SCALE = 1.0 / math.sqrt(DK)

_cache = {}


def _emit(tc, aps, s_len):
    from concourse import mybir

    nc = tc.nc
    bf16 = mybir.dt.bfloat16
    f32 = mybir.dt.float32

    nsb = s_len // SB  # s-blocks
    nkc = s_len // P  # key chunks
    jpb = SB // P  # 4 q-chunks per block

    query, key, value, Wq, bq, Wk, bk, Wv, bv, out = aps

    # weight APs rearranged to match the xbar layout: d = c*128 + p
    wq_ap = Wq.rearrange("(c p) k -> p c k", p=P)
    wk_ap = Wk.rearrange("(c p) k -> p c k", p=P)
    wv_ap = Wv.rearrange("(c p) k -> p c k", p=P)
    out_ap = out.rearrange("(nb j p) d -> nb p j d", p=P, j=jpb)

    import concourse.bass as bass
    from contextlib import ExitStack

    with ExitStack() as ctx:
        consts = ctx.enter_context(tc.tile_pool(name="consts", bufs=1))
        qkv = ctx.enter_context(tc.tile_pool(name="qkv", bufs=1))
        qtp = ctx.enter_context(tc.tile_pool(name="qt", bufs=nsb))
        ptp = ctx.enter_context(tc.tile_pool(name="pt", bufs=3))
        outp = ctx.enter_context(tc.tile_pool(name="outp", bufs=2))
        smallp = ctx.enter_context(tc.tile_pool(name="small", bufs=4))

        # --- constants ---
        wq_sb = consts.tile([P, CD, DK], bf16)
        wk_sb = consts.tile([P, CD, DK], bf16)
        wv_sb = consts.tile([P, CD, DV], bf16)
        nc.gpsimd.dma_start(out=wq_sb, in_=wq_ap)
        nc.gpsimd.dma_start(out=wk_sb, in_=wk_ap)
        nc.gpsimd.dma_start(out=wv_sb, in_=wv_ap)
        bq_sb = consts.tile([P, 1], f32)
        bk_sb = consts.tile([P, 1], f32)
        nc.sync.dma_start(out=bq_sb, in_=bq)
        nc.sync.dma_start(out=bk_sb, in_=bk)
        # bv broadcast across partitions (partition step 0)
        bv_bc = consts.tile([P, DV], f32)
        bv_bcast_ap = bass.AP(tensor=bv.tensor, offset=bv.offset, ap=[[0, P], [1, DV]])
        nc.gpsimd.dma_start(out=bv_bc, in_=bv_bcast_ap)

        # persistent per-core tensors
        kt_sb = qkv.tile([P, s_len], bf16)  # K^T  [dk, s]
        vp_sb = qkv.tile([P, nkc, DV + 1], bf16)  # V' natural [k % 128, chunk, dv+1]
        nc.vector.memset(vp_sb[:, :, DV : DV + 1], 1.0)
        qt_blocks = [qtp.tile([P, SB], bf16, tag="qt", name=f"qt{i}") for i in range(nsb)]

        with (
            tc.tile_pool(name="nat", bufs=6) as natp,
            tc.tile_pool(name="stage", bufs=2) as stagep,
            tc.tile_pool(name="ppsum", bufs=2, space="PSUM") as ppsum,
            tc.tile_pool(name="vpsum", bufs=2, space="PSUM") as vpsum,
        ):

            def load_transposed_block(src, sb):
                """Load s-block sb of src, return [128, CD, SB] bf16 (x^T staged)."""
                stage = stagep.tile([P, CD, SB], bf16, tag="stage")
                for j in range(jpb):
                    nat = natp.tile([P, D], bf16, tag="nat")
                    r0 = sb * SB + j * P
                    nc.gpsimd.dma_start(out=nat, in_=src[r0 : r0 + P, :])
                    nc.sync.dma_start(
                        out=stage[:, :, j * P : (j + 1) * P], in_=nat, transpose=True
                    )
                return stage

            # --- key -> K^T ---
            for sb in range(nsb):
                stage = load_transposed_block(key, sb)
                ps = ppsum.tile([P, SB], f32, tag="ppsum")
                for c in range(CD):
                    nc.tensor.matmul(
                        ps,
                        wk_sb[:, c, :],
                        stage[:, c, :],
                        start=(c == 0),
                        stop=(c == CD - 1),
                    )
                nc.vector.tensor_scalar_add(
                    kt_sb[:, sb * SB : (sb + 1) * SB], ps, bk_sb
                )

            # --- value -> V (natural) ---
            for sb in range(nsb):
                stage = load_transposed_block(value, sb)
                for j in range(jpb):
                    vps = vpsum.tile([P, DV], f32, tag="vpsum")
                    for c in range(CD):
                        nc.tensor.matmul(
                            vps,
                            stage[:, c, j * P : (j + 1) * P],
                            wv_sb[:, c, :],
                            start=(c == 0),
                            stop=(c == CD - 1),
                        )
                    nc.vector.tensor_copy(vp_sb[:, sb * jpb + j, 0:DV], vps)

            # --- query -> Q^T blocks ---
            for sb in range(nsb):
                stage = load_transposed_block(query, sb)
                ps = ppsum.tile([P, SB], f32, tag="ppsum")
                for c in range(CD):
                    nc.tensor.matmul(
                        ps,
                        wq_sb[:, c, :],
                        stage[:, c, :],
                        start=(c == 0),
                        stop=(c == CD - 1),
                    )
                nc.vector.tensor_scalar_add(qt_blocks[sb], ps, bq_sb)

        # --- attention ---
        with (
            tc.tile_pool(name="spsum", bufs=2, space="PSUM") as spsum,
            tc.tile_pool(name="opsum", bufs=jpb, space="PSUM") as opsum,
        ):
            for qb in range(nsb):
                ops = [opsum.tile([P, DV + 1], f32, tag="opsum", name=f"ops{qb}_{j}") for j in range(jpb)]
                for kc2 in range(nkc // 2):
                    sps = spsum.tile([P, 2 * SB], f32, tag="spsum")
                    for h in range(2):
                        kk = kc2 * 2 + h
                        nc.tensor.matmul(
                            sps[:, h * SB : (h + 1) * SB],
                            kt_sb[:, kk * P : (kk + 1) * P],
                            qt_blocks[qb],
                            start=True,
                            stop=True,
                        )
                    pt = ptp.tile([P, 2 * SB], bf16, tag="pt")
                    nc.scalar.activation(
                        pt, sps, mybir.ActivationFunctionType.Exp, scale=SCALE
                    )
                    for h in range(2):
                        kk = kc2 * 2 + h
                        for j in range(jpb):
                            nc.tensor.matmul(
                                ops[j],
                                pt[:, h * SB + j * P : h * SB + (j + 1) * P],
                                vp_sb[:, kk, :],
                                start=(kc2 == 0 and h == 0),
                                stop=(kc2 == nkc // 2 - 1 and h == 1),
                            )
                ostage = outp.tile([P, jpb, DV], f32, tag="ostage")
                for j in range(jpb):
                    recip = smallp.tile([P, 1], f32, tag="recip")
                    nc.vector.reciprocal(recip, ops[j][:, DV : DV + 1])
                    nc.vector.tensor_scalar_mul(ostage[:, j, :], ops[j][:, 0:DV], recip)
                    nc.vector.tensor_add(ostage[:, j, :], ostage[:, j, :], bv_bc)
                nc.sync.dma_start(out=out_ap[qb], in_=ostage)


def build(s_len=S):
    import concourse.tile as tile
    from concourse import bacc, mybir

    nc = bacc.Bacc(
        "TRN2",
        target_bir_lowering=False,
        debug=False,
        enable_asserts=False,
        num_devices=8,
    )
    f32 = mybir.dt.float32
    aps = [
        nc.dram_tensor("query", [s_len, D], f32, kind="ExternalInput").ap(),
        nc.dram_tensor("key", [s_len, D], f32, kind="ExternalInput").ap(),
        nc.dram_tensor("value", [s_len, D], f32, kind="ExternalInput").ap(),
        nc.dram_tensor("Wq", [D, DK], f32, kind="ExternalInput").ap(),
        nc.dram_tensor("bq", [DK, 1], f32, kind="ExternalInput").ap(),
        nc.dram_tensor("Wk", [D, DK], f32, kind="ExternalInput").ap(),
        nc.dram_tensor("bk", [DK, 1], f32, kind="ExternalInput").ap(),
        nc.dram_tensor("Wv", [D, DV], f32, kind="ExternalInput").ap(),
        nc.dram_tensor("bv", [1, DV], f32, kind="ExternalInput").ap(),
        nc.dram_tensor("out", [s_len, DV], f32, kind="ExternalOutput").ap(),
    ]
    with tile.TileContext(nc) as tc:
        _emit(tc, aps, s_len)
    nc.compile()
    return nc


def make_in_maps(inputs, s_len=S):
    weights = {
        "Wq": np.ascontiguousarray(inputs["Wq"], dtype=np.float32),
        "bq": np.ascontiguousarray(inputs["bq"], dtype=np.float32).reshape(DK, 1),
        "Wk": np.ascontiguousarray(inputs["Wk"], dtype=np.float32),
        "bk": np.ascontiguousarray(inputs["bk"], dtype=np.float32).reshape(DK, 1),
        "Wv": np.ascontiguousarray(inputs["Wv"], dtype=np.float32),
        "bv": np.ascontiguousarray(inputs["bv"], dtype=np.float32).reshape(1, DV),
    }
    in_maps = []
    for i in range(B):
        m = dict(weights)
        m["query"] = np.ascontiguousarray(inputs["query"][i], dtype=np.float32)
        m["key"] = np.ascontiguousarray(inputs["key"][i], dtype=np.float32)
        m["value"] = np.ascontiguousarray(inputs["value"][i], dtype=np.float32)
        in_maps.append(m)
    return in_maps


def kernel(**inputs):
    from concourse.bass_utils import run_bass_kernel_spmd

    if "nc" not in _cache:
        _cache["nc"] = build(S)
    nc = _cache["nc"]
    in_maps = make_in_maps(inputs, S)
    res = run_bass_kernel_spmd(nc, in_maps, core_ids=list(range(B)))
    return np.stack([r["out"] for r in res.results], axis=0)
